# revision 5
# baseline (speedup 1.0000x reference)
"""Trainium2 Bass kernel for DeepGate3-style attention segment pooling.

Computation (per tensor t in {hs, hf}):
    x = tok_t[member_idx]                  # [E, D] gather
    l = x @ w_t                            # [E]
    attn = softmax(l) within each segment  # segment_ids sorted, G segments
    out_t[g] = sum_{e in seg g} attn_e * x_e   # [G, D]

Strategy (8 cores, full I/O):
  - softmax shift-invariance: attn = exp(l)/segsum(exp(l)), with exp(l) a
    PER-NODE quantity exp(tok @ w) -> computed once per node, not per member.
  - shard segments across cores (contiguous ranges, since segment_ids sorted).
  - each core: computes exp(tok@w) for its N/8 node shard on PE, all-gathers
    the [N, 2] exp table; gathers member rows (bf16) + exp pairs from HBM via
    indirect DMA; segment-sums via small one-hot matmuls on PE
    (lhsT = X_chunk [128 mem, 128 D], rhs = S [128 mem, 8 segs] where
    S[j, w] = exp_j * (relseg_j == w)); z from one wide ones-matmul; divide.
  - segments are bin-packed on the host into 128-member chunks (<= 8 segs
    per chunk); each chunk owns its segments exclusively -> no PSUM
    accumulation across chunks; outputs land transposed [D, segcols] and the
    host transposes/scatters them back.
"""

import os

import numpy as np
import ml_dtypes

D = 128          # token dim (hard assumption throughout)
G_DEFAULT = 20000
NCORES_DEFAULT = 8
W_BIN = 8        # max segments packed per 128-member chunk
CHUNK = 128      # members per chunk == PE contraction dim
SUPER = 64       # chunks per super-group (gather/matmul batch)
DUMMY_REL = 15.0

_BF16 = ml_dtypes.bfloat16


def _pack_segments(sizes):
    """Bin-pack segments (member counts `sizes`) into chunks of capacity
    CHUNK with at most W_BIN segments per chunk. Returns list of lists of
    segment indices (local). Zero-size segments are excluded."""
    nz = np.nonzero(sizes > 0)[0]
    order = nz[np.argsort(sizes[nz], kind="stable")]  # ascending
    lo, hi = 0, len(order) - 1
    bins = []
    while lo <= hi:
        s = order[hi]
        if sizes[s] > CHUNK:
            raise ValueError(f"segment with {sizes[s]} members > {CHUNK}")
        cap = CHUNK - sizes[s]
        items = [s]
        hi -= 1
        while lo <= hi and len(items) < W_BIN and sizes[order[lo]] <= cap:
            items.append(order[lo])
            cap -= sizes[order[lo]]
            lo += 1
        bins.append(items)
    return bins


def _prep_host(member_idx, segment_ids, G, ncores):
    """Host-side sharding/layout. Returns per-core index arrays and the
    output assembly maps."""
    E = member_idx.shape[0]
    seg_start = np.searchsorted(segment_ids, np.arange(G + 1)).astype(np.int64)
    counts = np.diff(seg_start)
    segs_per_core = G // ncores
    assert segs_per_core * ncores == G

    per_core_bins = []
    for c in range(ncores):
        glo = c * segs_per_core
        bins = _pack_segments(counts[glo:glo + segs_per_core])
        per_core_bins.append((glo, bins))

    nchunk = max(len(b) for _, b in per_core_bins)
    nchunk = (nchunk + SUPER - 1) // SUPER * SUPER

    gidx = np.zeros((ncores, CHUNK, nchunk), np.int32)
    relseg = np.full((ncores, CHUNK, nchunk), DUMMY_REL, np.float32)
    out_cols = []   # per core: int array of packed output columns
    out_segs = []   # per core: global segment id per packed column
    for c, (glo, bins) in enumerate(per_core_bins):
        cols = []
        segs = []
        for k, items in enumerate(bins):
            p = 0
            for w, s in enumerate(items):
                a, b = seg_start[glo + s], seg_start[glo + s + 1]
                n = b - a
                gidx[c, p:p + n, k] = member_idx[a:b]
                relseg[c, p:p + n, k] = w
                p += n
                cols.append(k * W_BIN + w)
                segs.append(glo + s)
            assert p <= CHUNK
        out_cols.append(np.asarray(cols, np.int64))
        out_segs.append(np.asarray(segs, np.int64))
    return nchunk, gidx, relseg, out_cols, out_segs


def _build_bass(N, nchunk, ncores, x_mydt):
    import concourse.bacc as bacc
    import concourse.tile as tile
    import concourse.mybir as mybir
    from concourse import bass

    f32 = mybir.dt.float32
    i32 = mybir.dt.int32

    shard = N // ncores
    nsuper = nchunk // SUPER
    jcols = shard // CHUNK          # node columns per shard (per partition)
    piece = min(2048, shard)        # tokT load piece (f32 cols)
    kper = piece // CHUNK           # l-matmul chunks per piece

    nc = bacc.Bacc("TRN2", target_bir_lowering=False, debug=False,
                   num_devices=ncores)

    tok = {t: nc.dram_tensor(f"tok_{t}", [N, D], x_mydt, kind="ExternalInput")
           for t in ("hs", "hf")}
    tokt = {t: nc.dram_tensor(f"tokt_{t}", [D, shard], f32, kind="ExternalInput")
            for t in ("hs", "hf")}
    wts = {t: nc.dram_tensor(f"w_{t}", [D, 1], f32, kind="ExternalInput")
           for t in ("hs", "hf")}
    gidx_d = nc.dram_tensor("gidx", [CHUNK, nchunk], i32, kind="ExternalInput")
    relseg_d = nc.dram_tensor("relseg", [CHUNK, nchunk], x_mydt,
                              kind="ExternalInput")
    iota8_d = nc.dram_tensor("iota8", [CHUNK, W_BIN], x_mydt,
                             kind="ExternalInput")
    out_d = {t: nc.dram_tensor(f"out_{t}", [D, nchunk * W_BIN], f32,
                               kind="ExternalOutput") for t in ("hs", "hf")}

    with tile.TileContext(nc) as tc:
        with (
            tc.tile_pool(name="const", bufs=1) as constp,
            tc.tile_pool(name="tokt", bufs=2) as toktp,
            tc.tile_pool(name="exps", bufs=1) as expsp,
            tc.tile_pool(name="xg", bufs=2) as xgp,
            tc.tile_pool(name="sg", bufs=2) as sgp,
            tc.tile_pool(name="drain", bufs=2) as drainp,
            tc.tile_pool(name="psum", bufs=1, space="PSUM") as psump,
            tc.tile_pool(name="psumx", bufs=1, space="PSUM") as psumxp,
            tc.tile_pool(name="dram", bufs=1, space="DRAM") as dramp,
        ):
            # ---- constants / index loads -------------------------------
            gidx_sb = constp.tile([CHUNK, nchunk], i32, tag="gidx")
            nc.sync.dma_start(out=gidx_sb[:], in_=gidx_d.ap())
            relseg_sb = constp.tile([CHUNK, nchunk], x_mydt, tag="relseg")
            nc.sync.dma_start(out=relseg_sb[:], in_=relseg_d.ap())
            iota8_sb = constp.tile([CHUNK, W_BIN], x_mydt, tag="iota8")
            nc.sync.dma_start(out=iota8_sb[:], in_=iota8_d.ap())
            ones_sb = constp.tile([CHUNK, CHUNK], x_mydt, tag="ones")
            nc.vector.memset(ones_sb[:], 1.0)

            # ---- phase A: per-node logits -> exp table -> all-gather ----
            exp2_sb = expsp.tile([CHUNK, jcols, 2], f32, tag="exp2sb")
            for ti, t in enumerate(("hs", "hf")):
                w_sb = constp.tile([D, 1], f32, tag=f"w_{t}")
                nc.sync.dma_start(out=w_sb[:], in_=wts[t].ap())
                psum_l = psump.tile([CHUNK, jcols], f32, tag="psl")
                for b in range(shard // piece):
                    tt = toktp.tile([D, piece], f32, tag="tokt")
                    nc.sync.dma_start(
                        out=tt[:], in_=tokt[t].ap()[:, b * piece:(b + 1) * piece])
                    for k in range(kper):
                        j = b * kper + k
                        nc.tensor.matmul(
                            out=psum_l[:, j:j + 1],
                            lhsT=tt[:, k * CHUNK:(k + 1) * CHUNK],
                            rhs=w_sb[:], start=True, stop=True)
                nc.scalar.activation(
                    out=exp2_sb[:, :, ti:ti + 1].squeeze(2),
                    in_=psum_l[:],
                    func=mybir.ActivationFunctionType.Exp)

            exp2_shard = dramp.tile([shard, 2], f32, tag="e2shard")
            nc.sync.dma_start(
                out=exp2_shard[:].rearrange("(p j) t -> p (j t)", p=CHUNK),
                in_=exp2_sb[:].rearrange("p j t -> p (j t)"))
            exp2_full = dramp.tile([N, 2], f32, tag="e2full")
            nc.gpsimd.collective_compute(
                "AllGather", mybir.AluOpType.bypass,
                replica_groups=[list(range(ncores))],
                ins=[exp2_shard.opt()], outs=[exp2_full.opt()])

            # ---- phase B: gather + segment-reduce ----------------------
            for s in range(nsuper):
                idx_ap = gidx_sb[:, s * SUPER:(s + 1) * SUPER]
                xg = {}
                for t in ("hs", "hf"):
                    xg[t] = xgp.tile([CHUNK, SUPER, D], x_mydt, tag=f"x_{t}", name=f"x_{t}")
                    nc.gpsimd.indirect_dma_start(
                        out=xg[t][:], out_offset=None,
                        in_=tok[t].ap(),
                        in_offset=bass.IndirectOffsetOnAxis(ap=idx_ap, axis=0))
                expg = sgp.tile([CHUNK, SUPER, 2], f32, tag="expg")
                nc.gpsimd.indirect_dma_start(
                    out=expg[:], out_offset=None,
                    in_=exp2_full[:],
                    in_offset=bass.IndirectOffsetOnAxis(ap=idx_ap, axis=0))

                mask = sgp.tile([CHUNK, SUPER, W_BIN], x_mydt, tag="mask")
                nc.vector.tensor_tensor(
                    out=mask[:],
                    in0=relseg_sb[:, s * SUPER:(s + 1) * SUPER]
                        .unsqueeze(2).to_broadcast([CHUNK, SUPER, W_BIN]),
                    in1=iota8_sb[:].unsqueeze(1)
                        .to_broadcast([CHUNK, SUPER, W_BIN]),
                    op=mybir.AluOpType.is_equal)

                for ti, t in enumerate(("hs", "hf")):
                    s_t = sgp.tile([CHUNK, SUPER, W_BIN], x_mydt, tag=f"s_{t}")
                    nc.vector.tensor_tensor(
                        out=s_t[:], in0=mask[:],
                        in1=expg[:, :, ti:ti + 1]
                            .to_broadcast([CHUNK, SUPER, W_BIN]),
                        op=mybir.AluOpType.mult)

                    psum_x = psumxp.tile([CHUNK, SUPER * W_BIN], f32,
                                         tag=f"px_{t}")
                    for k in range(SUPER):
                        nc.tensor.matmul(
                            out=psum_x[:, k * W_BIN:(k + 1) * W_BIN],
                            lhsT=xg[t][:, k, :],
                            rhs=s_t[:, k, :], start=True, stop=True)
                    psum_z = psumxp.tile([CHUNK, SUPER * W_BIN], f32,
                                         tag=f"pz_{t}")
                    nc.tensor.matmul(
                        out=psum_z[:], lhsT=ones_sb[:],
                        rhs=s_t[:].rearrange("p a b -> p (a b)"),
                        start=True, stop=True)

                    zmax = drainp.tile([CHUNK, SUPER * W_BIN], f32,
                                       tag=f"zm_{t}")
                    nc.vector.tensor_scalar_max(
                        out=zmax[:], in0=psum_z[:], scalar1=1e-9)
                    zr = drainp.tile([CHUNK, SUPER * W_BIN], f32,
                                     tag=f"zr_{t}")
                    nc.vector.reciprocal(out=zr[:], in_=zmax[:])
                    osb = drainp.tile([CHUNK, SUPER * W_BIN], f32,
                                      tag=f"ob_{t}")
                    nc.vector.tensor_tensor(
                        out=osb[:], in0=psum_x[:], in1=zr[:],
                        op=mybir.AluOpType.mult)
                    nc.sync.dma_start(
                        out=out_d[t].ap()[:, s * SUPER * W_BIN:
                                          (s + 1) * SUPER * W_BIN],
                        in_=osb[:])
    nc.compile()
    return nc


def kernel(tf_hs, tf_hf, w_hs, w_hf, member_idx, segment_ids,
           _G=G_DEFAULT, _ncores=NCORES_DEFAULT, _trace=False, _sim=False):
    import concourse.mybir as mybir
    from concourse.bass_utils import run_bass_kernel_spmd

    tf_hs = np.asarray(tf_hs)
    tf_hf = np.asarray(tf_hf)
    w_hs = np.asarray(w_hs)
    w_hf = np.asarray(w_hf)
    member_idx = np.asarray(member_idx)
    segment_ids = np.asarray(segment_ids)

    N = tf_hs.shape[0]
    assert tf_hs.shape[1] == D
    ncores = _ncores
    G = _G
    shard = N // ncores

    x_np_dt = _BF16 if os.environ.get("KERNEL_XDTYPE", "bf16") == "bf16" \
        else np.float32
    x_mydt = mybir.dt.bfloat16 if x_np_dt is _BF16 else mybir.dt.float32

    nchunk, gidx, relseg, out_cols, out_segs = _prep_host(
        member_idx, segment_ids, G, ncores)

    nc = _build_bass(N, nchunk, ncores, x_mydt)

    tok_np = {"hs": np.ascontiguousarray(tf_hs.astype(x_np_dt)),
              "hf": np.ascontiguousarray(tf_hf.astype(x_np_dt))}
    w_np = {"hs": np.ascontiguousarray(w_hs.astype(np.float32).reshape(D, 1)),
            "hf": np.ascontiguousarray(w_hf.astype(np.float32).reshape(D, 1))}
    iota8 = np.broadcast_to(np.arange(W_BIN, dtype=np.float32), (CHUNK, W_BIN))
    iota8 = np.ascontiguousarray(iota8.astype(x_np_dt))

    in_maps = []
    for c in range(ncores):
        m = {}
        for t in ("hs", "hf"):
            m[f"tok_{t}"] = tok_np[t]
            src = tf_hs if t == "hs" else tf_hf
            sh = src[c * shard:(c + 1) * shard].astype(np.float32)
            # store maps node n = q*jcols + j <-> psum_l[q, j]; the matmul
            # gives psum_l[m, j] = sum_d tokt[d, j*128 + m] * w[d], so
            # tokt[d, j*128 + m] = shard_rows[m*jcols + j, d]
            m[f"tokt_{t}"] = np.ascontiguousarray(
                sh.reshape(CHUNK, shard // CHUNK, D).transpose(2, 1, 0)
                .reshape(D, shard))
            m[f"w_{t}"] = w_np[t]
        m["gidx"] = np.ascontiguousarray(gidx[c])
        m["relseg"] = np.ascontiguousarray(relseg[c].astype(x_np_dt))
        m["iota8"] = iota8
        in_maps.append(m)

    if _sim:
        from concourse.bass_interp import MultiCoreSim
        sim = MultiCoreSim(nc, num_cores=ncores, trace=False,
                           require_finite=False, require_nnan=False)
        for ci in range(ncores):
            core = sim.cores[ci]
            for name, arr in in_maps[ci].items():
                core.tensor(name)[:] = arr
        sim.simulate(check_with_hw=False)
        results = [{f"out_{t}": np.array(sim.cores[c].tensor(f"out_{t}"))
                    for t in ("hs", "hf")} for c in range(ncores)]
    else:
        res = run_bass_kernel_spmd(nc, in_maps, core_ids=list(range(ncores)),
                                   trace=_trace)
        results = res.results
        kernel.last_results = res

    hop = {t: np.zeros((G, D), np.float32) for t in ("hs", "hf")}
    for c in range(ncores):
        for t in ("hs", "hf"):
            o = results[c][f"out_{t}"]               # [D, nchunk*W_BIN]
            hop[t][out_segs[c]] = o[:, out_cols[c]].T
    return hop["hs"], hop["hf"]


kernel.last_results = None
